# revision 1
# baseline (speedup 1.0000x reference)
"""CausalTemporalAttnBlock Trainium2 kernel.

Problem: out = x + Wp @ attn(norm(x)) + bp, where norm is GroupNorm(1 group)
over (c,t,h,w) per batch, attention is causal over t, independent per (b,h,w).
Shapes: x (2, 512, 64, 32, 32) fp32; four (512,512) weights + biases.

Strategy (8 NeuronCores, zero communication except a 4-float AllReduce for
the GroupNorm stats):
  - core i handles batch i//4, h-rows [8*(i%4), 8*(i%4)+8), all w: 256 (h,w)
    locations per core.
  - Host folds gamma/beta/mean/rstd into the projection weights:
        q = r*(Aq @ x) + (cq - mu*r*uq),   Aq = wq*diag(gamma) (pre-scaled by
    1/sqrt(c) for q), uq = wq@gamma, cq = bq + wq@beta. mu, r=rstd computed
    on device (AllReduce of per-batch sum/sumsq across the 4 cores of each
    batch); the affine is applied at PSUM-eviction time.
  - Host re-lays the shard as [8 h-rows][512 c][64 t * 32 w] so every DMA is
    >=8KB contiguous.
  - Per h-row block (32 locs), per group of 8 locs: Q/K projections
    (c-on-partitions), V produced transposed (VT, t-on-partitions) directly
    by making x the stationary operand, scores computed transposed
    S^T = K^T Q (s-on-partitions) so softmax normalization is a ones-matmul
    and AV needs no transposes at all. No max-subtraction (scores are O(1);
    exp is safe in fp32). Causal mask applied as a 0/1 multiply after exp.
  - All big matmuls use float32r (full PE rate at N>=512, ~fp32 accuracy).
"""

import numpy as np

import concourse.bass as bass
import concourse.tile as tile
from concourse import bacc, mybir
from concourse.bass_utils import run_bass_kernel_spmd

P = 128
B, C, T, H, W = 2, 512, 64, 32, 32
NCORES = 8
HSH = H // 4          # 8 h-rows per core
CCH = C // P          # 4 c chunks
GRP = 8               # locations per attention group
NGRP = W // GRP       # 4 groups per block
EPS = 1e-6

f32 = mybir.dt.float32
f32r = mybir.dt.float32r
AX = mybir.AxisListType.X
ALU = mybir.AluOpType
AF = mybir.ActivationFunctionType


def build_nc(num_cores=NCORES, nblk=HSH, norm_n=None, replica_groups=None,
             reps=1, use_collective=True):
    if norm_n is None:
        norm_n = C * T * H * W
    if replica_groups is None:
        replica_groups = [[0, 1, 2, 3], [4, 5, 6, 7]]
    nc = bacc.Bacc("TRN2", target_bir_lowering=False, debug=False,
                   num_devices=num_cores)

    xs = nc.declare_dram_parameter("xs", [nblk, C, T * W], f32r, isOutput=False)
    wts = {}
    for nm in ("q", "k", "v", "p"):
        wts[nm] = nc.declare_dram_parameter(f"w{nm}t", [C, C], f32r,
                                            isOutput=False)
    ucol = nc.declare_dram_parameter("ucol", [P, 2 * CCH], f32, isOutput=False)
    ccol = nc.declare_dram_parameter("ccol", [P, 2 * CCH], f32, isOutput=False)
    uvrow = nc.declare_dram_parameter("uvrow", [1, C], f32, isOutput=False)
    cvrow = nc.declare_dram_parameter("cvrow", [1, C], f32, isOutput=False)
    bprow = nc.declare_dram_parameter("bprow", [1, C], f32r, isOutput=False)
    maskp = nc.declare_dram_parameter("maskt", [T, GRP * T], f32, isOutput=False)
    ones_col_f = nc.declare_dram_parameter("ones_col_f", [P, 1], f32, isOutput=False)
    ones_col_r = nc.declare_dram_parameter("ones_col_r", [P, 1], f32r, isOutput=False)
    ones_row_r = nc.declare_dram_parameter("ones_row_r", [1, C], f32r, isOutput=False)
    outp = nc.declare_dram_parameter("out", [nblk, C, T * W], f32,
                                     isOutput=True)
    cc_in = nc.dram_tensor("cc_in", [1, 2], f32)
    cc_out = nc.dram_tensor("cc_out", [1, 2], f32)

    with tile.TileContext(nc) as tc:
        with (
            tc.tile_pool(name="const", bufs=1) as const,
            tc.tile_pool(name="scal", bufs=1) as sc,
            tc.tile_pool(name="statp", bufs=2) as statp,
            tc.tile_pool(name="xpool", bufs=2) as xpool,
            tc.tile_pool(name="gpool", bufs=8) as gpool,
            tc.tile_pool(name="spool", bufs=2) as spool,
            tc.tile_pool(name="pp", bufs=3, space="PSUM") as pp,
            tc.tile_pool(name="pss", bufs=2, space="PSUM") as pss,
            tc.tile_pool(name="psm", bufs=1, space="PSUM") as psm,
        ):
            # ---------- constants ----------
            w_sb = {}
            for nm in ("q", "k", "v", "p"):
                for ci in range(CCH):
                    t = const.tile([P, C], f32r, tag=f"w{nm}{ci}")
                    nc.sync.dma_start(t[:], wts[nm][ci * P:(ci + 1) * P, :])
                    w_sb[nm, ci] = t
            ucol_sb = const.tile([P, 2 * CCH], f32, tag="ucol")
            nc.sync.dma_start(ucol_sb[:], ucol[:])
            ccol_sb = const.tile([P, 2 * CCH], f32, tag="ccol")
            nc.sync.dma_start(ccol_sb[:], ccol[:])
            uvrow_sb = const.tile([1, C], f32, tag="uvrow")
            nc.sync.dma_start(uvrow_sb[:], uvrow[:])
            cvrow_sb = const.tile([1, C], f32, tag="cvrow")
            nc.sync.dma_start(cvrow_sb[:], cvrow[:])
            bprow_sb = const.tile([1, C], f32r, tag="bprow")
            nc.sync.dma_start(bprow_sb[:], bprow[:])
            mask_sb = const.tile([T, GRP * T], f32, tag="maskt")
            nc.sync.dma_start(mask_sb[:], maskp[:])
            ocf_sb = const.tile([P, 1], f32, tag="ocf")
            nc.sync.dma_start(ocf_sb[:], ones_col_f[:])
            ocr_sb = const.tile([P, 1], f32r, tag="ocr")
            nc.sync.dma_start(ocr_sb[:], ones_col_r[:])
            orr_sb = const.tile([1, C], f32r, tag="orr")
            nc.sync.dma_start(orr_sb[:], ones_row_r[:])

            # repeat body for timing variants (reps>1)
            for _rep in range(reps):
                # ---------- stats ----------
                ssum = sc.tile([P, nblk * CCH], f32, tag="ssum")
                ssq = sc.tile([P, nblk * CCH], f32, tag="ssq")
                for blk in range(nblk):
                    for ci in range(CCH):
                        xt = statp.tile([P, T * W], f32, tag="xstat")
                        nc.sync.dma_start(
                            xt[:], xs[blk, ci * P:(ci + 1) * P, :].bitcast(f32))
                        i = blk * CCH + ci
                        nc.vector.reduce_sum(out=ssum[:, i:i + 1], in_=xt[:],
                                             axis=AX)
                        # tensor_tensor_reduce faults on this HW/runtime; square
                        # in place on ACT, then a plain DVE reduction
                        nc.scalar.activation(xt[:], xt[:], AF.Square)
                        nc.vector.reduce_sum(out=ssq[:, i:i + 1], in_=xt[:],
                                             axis=AX)
                st2 = sc.tile([P, 2], f32, tag="st2")
                nc.vector.reduce_sum(out=st2[:, 0:1], in_=ssum[:], axis=AX)
                nc.vector.reduce_sum(out=st2[:, 1:2], in_=ssq[:], axis=AX)
                ps_small = psm.tile([P, 512], f32, tag="psmall")
                nc.tensor.matmul(ps_small[0:1, 0:2], ocf_sb[:], st2[:],
                                 start=True, stop=True)
                st_sb = sc.tile([1, 2], f32, tag="st_sb")
                nc.vector.tensor_copy(st_sb[:], ps_small[0:1, 0:2])
                nc.gpsimd.dma_start(cc_in[:], st_sb[:])
                if use_collective:
                    nc.gpsimd.collective_compute(
                        "AllReduce", ALU.add, replica_groups=replica_groups,
                        ins=[cc_in[:]], outs=[cc_out[:]])
                else:
                    nc.gpsimd.dma_start(cc_out[:], cc_in[:])
                stg = sc.tile([1, 2], f32, tag="stg")
                nc.gpsimd.dma_start(stg[:], cc_out[:])

                mean = sc.tile([1, 1], f32, tag="mean")
                nc.scalar.activation(mean[:], stg[:, 0:1], AF.Copy,
                                     bias=0.0, scale=1.0 / norm_n)
                ex2 = sc.tile([1, 1], f32, tag="ex2")
                nc.scalar.activation(ex2[:], stg[:, 1:2], AF.Copy,
                                     bias=0.0, scale=1.0 / norm_n)
                msq = sc.tile([1, 1], f32, tag="msq")
                nc.scalar.activation(msq[:], mean[:], AF.Square)
                varp = sc.tile([1, 1], f32, tag="varp")
                nc.vector.tensor_scalar(varp[:], ex2[:], msq[:], EPS,
                                        ALU.subtract, ALU.add)
                sqv = sc.tile([1, 1], f32, tag="sqv")      # = 1/rstd
                nc.scalar.activation(sqv[:], varp[:], AF.Sqrt)
                rst = sc.tile([1, 1], f32, tag="rst")      # = rstd
                nc.vector.reciprocal(rst[:], sqv[:])
                rmu = sc.tile([1, 1], f32, tag="rmu")      # = rstd*mean
                nc.vector.tensor_scalar(rmu[:], mean[:], rst[:], None, ALU.mult)
                vals = sc.tile([1, 2], f32r, tag="vals")
                nc.vector.tensor_copy(vals[:, 0:1], rst[:])
                nc.vector.tensor_copy(vals[:, 1:2], rmu[:])
                # broadcast (rstd, rstd*mean) across 128 partitions via K=1 matmul
                nc.tensor.matmul(ps_small[:, 0:2], orr_sb[0:1, 0:P], vals[:],
                                 start=True, stop=True)
                rb = sc.tile([P, 2], f32, tag="rb")
                nc.vector.tensor_copy(rb[:], ps_small[:, 0:2])
                # per-(proj,chunk) eviction biases for q,k: D = ccol - rmu*ucol
                dcol = sc.tile([P, 2 * CCH], f32, tag="dcol")
                nc.vector.tensor_scalar(dcol[:], ucol_sb[:], rb[:, 1:2], None,
                                        ALU.mult)
                nc.vector.tensor_sub(dcol[:], ccol_sb[:], dcol[:])
                # VT rank-1 row: dvr = (cvrow - rmu*uvrow) / rstd
                tv0 = sc.tile([1, C], f32, tag="tv0")
                nc.vector.tensor_scalar(tv0[:], uvrow_sb[:], rmu[:], None,
                                        ALU.mult)
                nc.vector.tensor_sub(tv0[:], cvrow_sb[:], tv0[:])
                dvr = sc.tile([1, C], f32r, tag="dvr")
                nc.vector.tensor_scalar(dvr[:], tv0[:], sqv[:], None, ALU.mult)

                # ---------- main blocks ----------
                for blk in range(nblk):
                    xb = []
                    for ci in range(CCH):
                        t = xpool.tile([P, T * W], f32r, tag=f"xb{ci}")
                        nc.sync.dma_start(t[:], xs[blk, ci * P:(ci + 1) * P, :])
                        xb.append(t)

                    def xgrp(ci, w0, n=GRP):
                        # [128, w(n) x t(64)] view of group cols, w-major
                        return xb[ci][:].rearrange(
                            "p (t w) -> p w t", w=W)[:, w0:w0 + n, :]

                    def xloc(ci, w):
                        # [128, t(64)] stationary view for VT production
                        return xb[ci][:].rearrange(
                            "p (t w) -> p w t", w=W)[:, w, :]

                    for g in range(NGRP):
                        w0 = g * GRP
                        # ---- Q, K projections: psum[co, (t,w)] over ci ----
                        qk = {}
                        for pi, nm in enumerate(("q", "k")):
                            for co in range(CCH):
                                ps = pp.tile([P, 512], f32, tag="pp")
                                for ci in range(CCH):
                                    nc.tensor.matmul(
                                        ps[:], w_sb[nm, ci][:, co * P:(co + 1) * P],
                                        xgrp(ci, w0), start=(ci == 0),
                                        stop=(ci == CCH - 1))
                                t = gpool.tile([P, 512], f32, tag=f"{nm}g")
                                d = pi * CCH + co
                                nc.vector.tensor_scalar(
                                    t[:], ps[:], rb[:, 0:1], dcol[:, d:d + 1],
                                    ALU.mult, ALU.add)
                                qk[nm, co] = t

                        # ---- VT: per loc, [64 s, 512 co] ----
                        vt = []
                        for w in range(GRP):
                            ps = pss.tile([T, 512], f32, tag="ppv")
                            for ci in range(CCH):
                                nc.tensor.matmul(ps[:], xloc(ci, w0 + w),
                                                 w_sb["v", ci][:],
                                                 start=(ci == 0), stop=False)
                            nc.tensor.matmul(ps[:], orr_sb[0:1, 0:T], dvr[:],
                                             start=False, stop=True)
                            t = gpool.tile([T, 512], f32r, tag="vtg")
                            nc.scalar.activation(t[:], ps[:], AF.Copy, bias=0.0,
                                                 scale=rb[0:T, 0:1])
                            vt.append(t)

                        # ---- scores S^T[s, (w,t)] ----
                        # one bank holds 8 independent accumulation chains, so
                        # zero it explicitly (PSUM start=True zeroes the whole
                        # 2KB bank, clobbering sibling chains) and accumulate
                        # with start=False onto the memset zeros
                        ps_s = psm.tile([T, 512], f32, tag="pss")
                        nc.vector.memset(ps_s[:], 0.0)
                        for w in range(GRP):
                            for ci in range(CCH):
                                kl = qk["k", ci][:, w * T:(w + 1) * T]
                                ql = qk["q", ci][:, w * T:(w + 1) * T]
                                nc.tensor.matmul(ps_s[:, w * T:(w + 1) * T],
                                                 kl, ql, start=False,
                                                 stop=(ci == CCH - 1),
                                                 skip_group_check=True)
                        # ---- softmax (no max-subtraction) ----
                        pexp = spool.tile([T, 512], f32r, tag="pexp")
                        nc.scalar.activation(pexp[:], ps_s[:], AF.Exp)
                        pm = spool.tile([T, 512], f32r, tag="pmask")
                        nc.vector.tensor_mul(pm[:], pexp[:].bitcast(f32),
                                             mask_sb[:])
                        ps_sum = psm.tile([1, 512], f32, tag="psum_s")
                        nc.tensor.matmul(ps_sum[:], ocr_sb[0:T, :], pm[:],
                                         start=True, stop=True)
                        rs = spool.tile([1, 512], f32r, tag="rs")
                        with nc.allow_low_precision(
                                reason="float32r is full fp32 storage"):
                            nc.vector.reciprocal(rs[:], ps_sum[:])
                        ps_rb = psm.tile([T, 512], f32, tag="psmall")
                        nc.tensor.matmul(ps_rb[:], orr_sb[0:1, 0:T], rs[:],
                                         start=True, stop=True)
                        pn = spool.tile([T, 512], f32r, tag="pn")
                        nc.vector.tensor_mul(pn[:], pm[:].bitcast(f32), ps_rb[:])

                        # ---- AV: O[c,(w,t)] ----
                        og = []
                        for ch in range(CCH):
                            ps_o = pp.tile([P, 512], f32, tag="pp")
                            nc.vector.memset(ps_o[:], 0.0)
                            for w in range(GRP):
                                lhsT = vt[w][:, ch * P:(ch + 1) * P]
                                nc.tensor.matmul(ps_o[:, w * T:(w + 1) * T],
                                                 lhsT, pn[:, w * T:(w + 1) * T],
                                                 start=False, stop=True,
                                                 skip_group_check=True)
                            t = gpool.tile([P, 512], f32r, tag="og")
                            nc.scalar.copy(t[:], ps_o[:])
                            og.append(t)

                        # ---- P-projection + bias (rank-1) + residual ----
                        for co in range(CCH):
                            ps = pp.tile([P, 512], f32, tag="pp")
                            for ci in range(CCH):
                                nc.tensor.matmul(
                                    ps[:], w_sb["p", ci][:, co * P:(co + 1) * P],
                                    og[ci][:], start=(ci == 0), stop=False)
                            nc.tensor.matmul(
                                ps[:], bprow_sb[:, co * P:(co + 1) * P],
                                orr_sb[:, 0:512], start=False, stop=True)
                            ps3 = ps[:].rearrange("p (w t) -> p w t", w=GRP)
                            xsl = xgrp(co, w0)
                            nc.vector.tensor_add(xsl, ps3, xsl.bitcast(f32))

                    for ci in range(CCH):
                        nc.sync.dma_start(outp[blk, ci * P:(ci + 1) * P, :],
                                          xb[ci][:].bitcast(f32))
    nc.compile()
    return nc


def host_prep(gamma, beta, wq, bq, wk, bk, wv, bv, wp, bp):
    """Fold gamma/beta into weights; build all constant tensors."""
    s = 1.0 / np.sqrt(np.float32(C))
    g = gamma.astype(np.float64)

    def fold(w, bias, scale):
        a = (w.astype(np.float64) * g[None, :]) * scale      # (co, ci)
        u = (w.astype(np.float64) @ g) * scale               # (co,)
        c0 = (bias.astype(np.float64) + w.astype(np.float64) @
              beta.astype(np.float64)) * scale
        return (np.ascontiguousarray(a.T.astype(np.float32)),
                u.astype(np.float32), c0.astype(np.float32))

    aqt, uq, cq = fold(wq, bq, s)
    akt, uk, ck = fold(wk, bk, 1.0)
    avt, uv, cv = fold(wv, bv, 1.0)
    apt = np.ascontiguousarray(wp.T.astype(np.float32))

    ucol = np.empty((P, 2 * CCH), np.float32)
    ccol = np.empty((P, 2 * CCH), np.float32)
    for pi, (u, c0) in enumerate(((uq, cq), (uk, ck))):
        for ch in range(CCH):
            ucol[:, pi * CCH + ch] = u[ch * P:(ch + 1) * P]
            ccol[:, pi * CCH + ch] = c0[ch * P:(ch + 1) * P]

    maskt = np.tile(np.triu(np.ones((T, T), np.float32)), (1, GRP))
    consts = {
        "wqt": aqt, "wkt": akt, "wvt": avt, "wpt": apt,
        "ucol": ucol, "ccol": ccol,
        "uvrow": uv[None, :].copy(), "cvrow": cv[None, :].copy(),
        "bprow": bp.astype(np.float32)[None, :].copy(),
        "maskt": np.ascontiguousarray(maskt),
        "ones_col_f": np.ones((P, 1), np.float32),
        "ones_col_r": np.ones((P, 1), np.float32),
        "ones_row_r": np.ones((1, C), np.float32),
    }
    return consts


_NC_CACHE = {}


def kernel(x, gamma, beta, wq, bq, wk, bk, wv, bv, wp, bp):
    x = np.asarray(x, np.float32)
    args = [np.asarray(a, np.float32) for a in
            (gamma, beta, wq, bq, wk, bk, wv, bv, wp, bp)]
    consts = host_prep(*args)

    if "nc" not in _NC_CACHE:
        _NC_CACHE["nc"] = build_nc()
    nc = _NC_CACHE["nc"]

    in_maps = []
    for core in range(NCORES):
        b, hg = core // 4, core % 4
        shard = x[b, :, :, hg * HSH:(hg + 1) * HSH, :]        # (C,T,HSH,W)
        shard = np.ascontiguousarray(
            shard.transpose(2, 0, 1, 3)).reshape(HSH, C, T * W)
        in_maps.append({"xs": shard, **consts})

    global _last_in_maps
    _last_in_maps = in_maps
    res = run_bass_kernel_spmd(nc, in_maps, list(range(NCORES)))

    out = np.empty((B, C, T, H, W), np.float32)
    for core in range(NCORES):
        b, hg = core // 4, core % 4
        o = res.results[core]["out"].reshape(HSH, C, T, W)
        out[b, :, :, hg * HSH:(hg + 1) * HSH, :] = o.transpose(1, 2, 0, 3)
    return out



# revision 2
# speedup vs baseline: 1.8945x; 1.8945x over previous
"""CausalTemporalAttnBlock Trainium2 kernel.

Problem: out = x + Wp @ attn(norm(x)) + bp, where norm is GroupNorm(1 group)
over (c,t,h,w) per batch, attention is causal over t, independent per (b,h,w).
Shapes: x (2, 512, 64, 32, 32) fp32; four (512,512) weights + biases.

Strategy (8 NeuronCores, zero communication except a 8-byte AllReduce for
the GroupNorm stats):
  - core i handles batch i//4, h-rows [8*(i%4), 8*(i%4)+8), all w: 256 (h,w)
    locations per core.
  - Whole matmul datapath in bf16 (fp32 PSUM accumulation): full-rate PE
    streaming (fp32 is half rate) and fast weight load. Host quantizes x
    and the folded weights to bf16 (RNE); error stays ~1e-3 vs the 2e-2
    budget.
  - Host folds gamma/beta into the projection weights:
        q = r*(Aq @ x) + (cq - mu*r*uq),  Aq = wq*diag(gamma) (q pre-scaled
    by 1/sqrt(c)), uq = wq@gamma, cq = bq + wq@beta; same for k. The V-path
    affine is folded all the way into the P-projection eviction:
        out = x + r*(Wp @ attn @ (Av x)^T) + (Wp @ dv + bp), dv = cv - mu*r*uv
    (softmax rows sum to 1, so the V bias passes through attention as a
    per-channel constant). mu, r=rstd computed on device (AllReduce of
    per-batch sum/sumsq across the 4 cores of each batch).
  - Host re-lays the shard w-major: [8 h-rows][512 c][32 w * 64 t], so one
    attention group (8 w-locations) is a contiguous 512-column slice.
  - Locations are processed in PAIRS sharing the 128-wide stationary
    operand: VT for 2 locations in one matmul (x-pair stationary, Wv
    moving), scores S^T = K^T Q per pair as one [128,128] matmul (cross
    terms masked off with the causal mask), and AV as a full-K [128,128]
    matmul per pair (masked zeros in attn kill the cross contributions).
  - No max-subtraction in softmax (scores are O(1)); causal+pair mask is a
    0/1 multiply after exp; normalization by 1/rowsum via ones-matmul
    reductions/broadcasts on the PE.
"""

import numpy as np
import ml_dtypes

import concourse.bass as bass
import concourse.tile as tile
from concourse import bacc, mybir
from concourse.bass_utils import run_bass_kernel_spmd

P = 128
B, C, T, H, W = 2, 512, 64, 32, 32
NCORES = 8
HSH = H // 4          # 8 h-rows per core
CCH = C // P          # 4 c chunks
GRP = 8               # locations per attention group
NGRP = W // GRP       # 4 groups per h-row block
NPR = GRP // 2        # 4 location-pairs per group
WT = W * T            # 2048 free columns per (h-row, c) plane
EPS = 1e-6

f32 = mybir.dt.float32
bf16 = mybir.dt.bfloat16
AX = mybir.AxisListType.X
ALU = mybir.AluOpType
AF = mybir.ActivationFunctionType
BF = ml_dtypes.bfloat16


def build_nc(num_cores=NCORES, nblk=HSH, norm_n=None, replica_groups=None,
             use_collective=True):
    if norm_n is None:
        norm_n = C * T * H * W
    if replica_groups is None:
        replica_groups = [[0, 1, 2, 3], [4, 5, 6, 7]]
    nc = bacc.Bacc("TRN2", target_bir_lowering=False, debug=False,
                   num_devices=num_cores)

    xs = nc.declare_dram_parameter("xs", [nblk, C, WT], bf16, isOutput=False)
    wts = {}
    for nm in ("q", "k", "v", "p"):
        wts[nm] = nc.declare_dram_parameter(f"w{nm}t", [C, C], bf16,
                                            isOutput=False)
    ucol = nc.declare_dram_parameter("ucol", [P, 3 * CCH], f32, isOutput=False)
    ccol = nc.declare_dram_parameter("ccol", [P, 3 * CCH], f32, isOutput=False)
    bpcol = nc.declare_dram_parameter("bpcol", [P, CCH], f32, isOutput=False)
    maskp = nc.declare_dram_parameter("maskt", [P, NPR * P], bf16,
                                      isOutput=False)
    ones_col_f = nc.declare_dram_parameter("ones_col_f", [P, 1], f32,
                                           isOutput=False)
    ones_row_f = nc.declare_dram_parameter("ones_row_f", [1, P], f32,
                                           isOutput=False)
    ones_col_b = nc.declare_dram_parameter("ones_col_b", [P, 1], bf16,
                                           isOutput=False)
    ones_row_b = nc.declare_dram_parameter("ones_row_b", [1, P], bf16,
                                           isOutput=False)
    outp = nc.declare_dram_parameter("out", [nblk, C, WT], f32, isOutput=True)
    cc_in = nc.dram_tensor("cc_in", [1, 2], f32)
    cc_out = nc.dram_tensor("cc_out", [1, 2], f32)

    with tile.TileContext(nc) as tc:
        with (
            tc.tile_pool(name="const", bufs=1) as const,
            tc.tile_pool(name="scal", bufs=1) as sc,
            tc.tile_pool(name="statp", bufs=4) as statp,
            tc.tile_pool(name="sqp", bufs=2) as sqp,
            tc.tile_pool(name="xpool", bufs=2) as xpool,
            tc.tile_pool(name="gpool", bufs=8) as gpool,
            tc.tile_pool(name="spool", bufs=2) as spool,
            tc.tile_pool(name="opool", bufs=4) as opool,
            tc.tile_pool(name="pp", bufs=3, space="PSUM") as pp,
            tc.tile_pool(name="pss", bufs=2, space="PSUM") as pss,
            tc.tile_pool(name="scp", bufs=2, space="PSUM") as scp,
            tc.tile_pool(name="psm", bufs=1, space="PSUM") as psm,
        ):
            # ---------- constants ----------
            w_sb = {}
            for nm in ("q", "k", "v", "p"):
                for ci in range(CCH):
                    t = const.tile([P, C], bf16, tag=f"w{nm}{ci}")
                    nc.sync.dma_start(t[:], wts[nm][ci * P:(ci + 1) * P, :])
                    w_sb[nm, ci] = t
            ucol_sb = const.tile([P, 3 * CCH], f32, tag="ucol")
            nc.sync.dma_start(ucol_sb[:], ucol[:])
            ccol_sb = const.tile([P, 3 * CCH], f32, tag="ccol")
            nc.sync.dma_start(ccol_sb[:], ccol[:])
            bpcol_sb = const.tile([P, CCH], f32, tag="bpcol")
            nc.sync.dma_start(bpcol_sb[:], bpcol[:])
            mask_sb = const.tile([P, NPR * P], bf16, tag="maskt")
            nc.sync.dma_start(mask_sb[:], maskp[:])
            ocf_sb = const.tile([P, 1], f32, tag="ocf")
            nc.sync.dma_start(ocf_sb[:], ones_col_f[:])
            orf_sb = const.tile([1, P], f32, tag="orf")
            nc.sync.dma_start(orf_sb[:], ones_row_f[:])
            ocb_sb = const.tile([P, 1], bf16, tag="ocb")
            nc.sync.dma_start(ocb_sb[:], ones_col_b[:])
            orb_sb = const.tile([1, P], bf16, tag="orb")
            nc.sync.dma_start(orb_sb[:], ones_row_b[:])

            # ---------- stats (sum / sumsq over the whole shard) ----------
            ssum = sc.tile([P, nblk * CCH], f32, tag="ssum")
            ssq = sc.tile([P, nblk * CCH], f32, tag="ssq")
            for blk in range(nblk):
                for ci in range(CCH):
                    xt = statp.tile([P, WT], bf16, tag="xstat")
                    nc.sync.dma_start(xt[:], xs[blk, ci * P:(ci + 1) * P, :])
                    i = blk * CCH + ci
                    nc.vector.reduce_sum(out=ssum[:, i:i + 1], in_=xt[:],
                                         axis=AX)
                    sq = sqp.tile([P, WT], bf16, tag="sq")
                    nc.scalar.activation(sq[:], xt[:], AF.Square)
                    nc.vector.reduce_sum(out=ssq[:, i:i + 1], in_=sq[:],
                                         axis=AX)
            st2 = sc.tile([P, 2], f32, tag="st2")
            nc.vector.reduce_sum(out=st2[:, 0:1], in_=ssum[:], axis=AX)
            nc.vector.reduce_sum(out=st2[:, 1:2], in_=ssq[:], axis=AX)
            ps_small = psm.tile([P, 512], f32, tag="psm")
            nc.tensor.matmul(ps_small[0:1, 0:2], ocf_sb[:], st2[:],
                             start=True, stop=True)
            st_sb = sc.tile([1, 2], f32, tag="st_sb")
            nc.vector.tensor_copy(st_sb[:], ps_small[0:1, 0:2])
            nc.gpsimd.dma_start(cc_in[:], st_sb[:])
            if use_collective:
                nc.gpsimd.collective_compute(
                    "AllReduce", ALU.add, replica_groups=replica_groups,
                    ins=[cc_in[:]], outs=[cc_out[:]])
            else:
                nc.gpsimd.dma_start(cc_out[:], cc_in[:])
            stg = sc.tile([1, 2], f32, tag="stg")
            nc.gpsimd.dma_start(stg[:], cc_out[:])

            mean = sc.tile([1, 1], f32, tag="mean")
            nc.scalar.activation(mean[:], stg[:, 0:1], AF.Copy,
                                 bias=0.0, scale=1.0 / norm_n)
            ex2 = sc.tile([1, 1], f32, tag="ex2")
            nc.scalar.activation(ex2[:], stg[:, 1:2], AF.Copy,
                                 bias=0.0, scale=1.0 / norm_n)
            msq = sc.tile([1, 1], f32, tag="msq")
            nc.scalar.activation(msq[:], mean[:], AF.Square)
            varp = sc.tile([1, 1], f32, tag="varp")
            nc.vector.tensor_scalar(varp[:], ex2[:], msq[:], EPS,
                                    ALU.subtract, ALU.add)
            sqv = sc.tile([1, 1], f32, tag="sqv")      # = 1/rstd
            nc.scalar.activation(sqv[:], varp[:], AF.Sqrt)
            rst = sc.tile([1, 1], f32, tag="rst")      # = rstd
            nc.vector.reciprocal(rst[:], sqv[:])
            rmu = sc.tile([1, 1], f32, tag="rmu")      # = rstd*mean
            nc.vector.tensor_scalar(rmu[:], mean[:], rst[:], None, ALU.mult)
            vals = sc.tile([1, 2], f32, tag="vals")
            nc.vector.tensor_copy(vals[:, 0:1], rst[:])
            nc.vector.tensor_copy(vals[:, 1:2], rmu[:])
            # broadcast (rstd, rstd*mean) across 128 partitions via K=1 matmul
            ps_b = psm.tile([P, 512], f32, tag="psm")
            nc.tensor.matmul(ps_b[:, 0:2], orf_sb[:], vals[:],
                             start=True, stop=True)
            rb = sc.tile([P, 2], f32, tag="rb")
            nc.vector.tensor_copy(rb[:], ps_b[:, 0:2])
            # per-(proj,chunk) eviction biases for q,k,v: D = ccol - rmu*ucol
            dcol = sc.tile([P, 3 * CCH], f32, tag="dcol")
            nc.vector.tensor_scalar(dcol[:], ucol_sb[:], rb[:, 1:2], None,
                                    ALU.mult)
            nc.vector.tensor_sub(dcol[:], ccol_sb[:], dcol[:])
            # dp = Wp @ dv + bp  (per-channel constant added at P-eviction)
            dvb = sc.tile([P, CCH], bf16, tag="dvb")
            nc.vector.tensor_copy(dvb[:], dcol[:, 2 * CCH:3 * CCH])
            ps_dp = psm.tile([P, 512], f32, tag="psm")
            for co in range(CCH):
                for ci in range(CCH):
                    nc.tensor.matmul(
                        ps_dp[:, co:co + 1],
                        w_sb["p", ci][:, co * P:(co + 1) * P],
                        dvb[:, ci:ci + 1], start=(co == 0 and ci == 0),
                        stop=(ci == CCH - 1), skip_group_check=True)
            dp = sc.tile([P, CCH], f32, tag="dp")
            nc.vector.tensor_add(dp[:], ps_dp[:, 0:CCH], bpcol_sb[:])

            # ---------- main blocks ----------
            for blk in range(nblk):
                xb = []
                for ci in range(CCH):
                    t = xpool.tile([P, WT], bf16, tag=f"xb{ci}")
                    nc.sync.dma_start(t[:], xs[blk, ci * P:(ci + 1) * P, :])
                    xb.append(t)

                for g in range(NGRP):
                    cs = g * GRP * T          # 512-col slice of this group
                    # ---- Q, K projections: psum[co, (w,t)] over ci ----
                    qk = {}
                    for pi, nm in enumerate(("q", "k")):
                        for co in range(CCH):
                            ps = pp.tile([P, 512], f32, tag="pp")
                            for ci in range(CCH):
                                nc.tensor.matmul(
                                    ps[:], w_sb[nm, ci][:, co * P:(co + 1) * P],
                                    xb[ci][:, cs:cs + 512], start=(ci == 0),
                                    stop=(ci == CCH - 1))
                            t = gpool.tile([P, 512], bf16, tag=f"{nm}g")
                            d = pi * CCH + co
                            nc.vector.tensor_scalar(
                                t[:], ps[:], rb[:, 0:1], dcol[:, d:d + 1],
                                ALU.mult, ALU.add)
                            qk[nm, co] = t

                    # ---- VT (raw): per loc PAIR, [128 (2w,s), 512 co] ----
                    vtp = []
                    for p in range(NPR):
                        ps = pss.tile([P, 512], f32, tag="ppv")
                        for ci in range(CCH):
                            nc.tensor.matmul(
                                ps[:], xb[ci][:, cs + p * P:cs + (p + 1) * P],
                                w_sb["v", ci][:], start=(ci == 0),
                                stop=(ci == CCH - 1))
                        t = gpool.tile([P, 512], bf16, tag="vtg")
                        nc.scalar.copy(t[:], ps[:])
                        vtp.append(t)

                    # ---- scores S^T[(2w,s), (2w,t)] per pair ----
                    # 4 pair-chains share one PSUM bank: the very first matmul
                    # start=True zeroes the bank, later chains' first matmuls
                    # overwrite (has_written cleared) and accumulate over ci.
                    ps_s = scp.tile([P, 512], f32, tag="pss")
                    for p in range(NPR):
                        for ci in range(CCH):
                            nc.tensor.matmul(
                                ps_s[:, p * P:(p + 1) * P],
                                qk["k", ci][:, p * P:(p + 1) * P],
                                qk["q", ci][:, p * P:(p + 1) * P],
                                start=(p == 0 and ci == 0),
                                stop=(ci == CCH - 1), skip_group_check=True)
                    # ---- softmax (no max-subtraction) ----
                    pexp = spool.tile([P, 512], bf16, tag="pexp")
                    nc.scalar.activation(pexp[:], ps_s[:], AF.Exp)
                    pm = spool.tile([P, 512], bf16, tag="pmask")
                    nc.vector.tensor_mul(pm[:], pexp[:], mask_sb[:])
                    ps_sum = psm.tile([1, 512], f32, tag="psm")
                    nc.tensor.matmul(ps_sum[:], ocb_sb[:], pm[:],
                                     start=True, stop=True)
                    rs = spool.tile([1, 512], bf16, tag="rs")
                    with nc.allow_low_precision(
                            reason="bf16 softmax denom fine at 2e-2 target"):
                        nc.vector.reciprocal(rs[:], ps_sum[:])
                    ps_rb = psm.tile([P, 512], f32, tag="psm")
                    nc.tensor.matmul(ps_rb[:], orb_sb[:], rs[:],
                                     start=True, stop=True)
                    pn = spool.tile([P, 512], bf16, tag="pn")
                    nc.vector.tensor_mul(pn[:], pm[:], ps_rb[:])

                    # ---- AV: O[c,(2w,t)] per pair, full-K (mask zeros kill
                    # the cross-location contributions) ----
                    og = []
                    for ch in range(CCH):
                        ps_o = pp.tile([P, 512], f32, tag="pp")
                        for p in range(NPR):
                            nc.tensor.matmul(
                                ps_o[:, p * P:(p + 1) * P],
                                vtp[p][:, ch * P:(ch + 1) * P],
                                pn[:, p * P:(p + 1) * P],
                                start=(p == 0), stop=True,
                                skip_group_check=True)
                        t = gpool.tile([P, 512], bf16, tag="og")
                        nc.vector.tensor_copy(t[:], ps_o[:])
                        og.append(t)

                    # ---- P-projection + affine + residual ----
                    for co in range(CCH):
                        ps = pp.tile([P, 512], f32, tag="pp")
                        for ci in range(CCH):
                            nc.tensor.matmul(
                                ps[:], w_sb["p", ci][:, co * P:(co + 1) * P],
                                og[ci][:], start=(ci == 0),
                                stop=(ci == CCH - 1))
                        slab = opool.tile([P, 512], f32, tag="oslab")
                        nc.vector.tensor_scalar(
                            slab[:], ps[:], rb[:, 0:1], dp[:, co:co + 1],
                            ALU.mult, ALU.add)
                        nc.vector.tensor_add(slab[:], slab[:],
                                             xb[co][:, cs:cs + 512])
                        nc.sync.dma_start(
                            outp[blk, co * P:(co + 1) * P, cs:cs + 512],
                            slab[:])
    nc.compile()
    return nc


def host_prep(gamma, beta, wq, bq, wk, bk, wv, bv, wp, bp):
    """Fold gamma/beta into weights; build all constant tensors."""
    s = 1.0 / np.sqrt(np.float32(C))
    g = gamma.astype(np.float64)

    def fold(w, bias, scale):
        a = (w.astype(np.float64) * g[None, :]) * scale      # (co, ci)
        u = (w.astype(np.float64) @ g) * scale               # (co,)
        c0 = (bias.astype(np.float64) + w.astype(np.float64) @
              beta.astype(np.float64)) * scale
        return (np.ascontiguousarray(a.T).astype(BF),
                u.astype(np.float32), c0.astype(np.float32))

    aqt, uq, cq = fold(wq, bq, s)
    akt, uk, ck = fold(wk, bk, 1.0)
    avt, uv, cv = fold(wv, bv, 1.0)
    apt = np.ascontiguousarray(wp.T.astype(np.float32)).astype(BF)

    ucol = np.empty((P, 3 * CCH), np.float32)
    ccol = np.empty((P, 3 * CCH), np.float32)
    for pi, (u, c0) in enumerate(((uq, cq), (uk, ck), (uv, cv))):
        for ch in range(CCH):
            ucol[:, pi * CCH + ch] = u[ch * P:(ch + 1) * P]
            ccol[:, pi * CCH + ch] = c0[ch * P:(ch + 1) * P]
    bpcol = np.empty((P, CCH), np.float32)
    for ch in range(CCH):
        bpcol[:, ch] = bp[ch * P:(ch + 1) * P]

    # pair mask [128, 4*128]: diag 64x64 halves get causal triu (s<=t),
    # off-diag (cross-location) halves are zero; identical per pair.
    tri = np.triu(np.ones((T, T), np.float32))
    blkm = np.zeros((P, P), np.float32)
    blkm[0:T, 0:T] = tri
    blkm[T:2 * T, T:2 * T] = tri
    maskt = np.tile(blkm, (1, NPR))

    consts = {
        "wqt": aqt, "wkt": akt, "wvt": avt, "wpt": apt,
        "ucol": ucol, "ccol": ccol, "bpcol": bpcol,
        "maskt": maskt.astype(BF),
        "ones_col_f": np.ones((P, 1), np.float32),
        "ones_row_f": np.ones((1, P), np.float32),
        "ones_col_b": np.ones((P, 1), BF),
        "ones_row_b": np.ones((1, P), BF),
    }
    return consts


_NC_CACHE = {}


def kernel(x, gamma, beta, wq, bq, wk, bk, wv, bv, wp, bp):
    x = np.asarray(x, np.float32)
    args = [np.asarray(a, np.float32) for a in
            (gamma, beta, wq, bq, wk, bk, wv, bv, wp, bp)]
    consts = host_prep(*args)

    if "nc" not in _NC_CACHE:
        _NC_CACHE["nc"] = build_nc()
    nc = _NC_CACHE["nc"]

    in_maps = []
    for core in range(NCORES):
        b, hg = core // 4, core % 4
        shard = x[b, :, :, hg * HSH:(hg + 1) * HSH, :]        # (C,T,HSH,W)
        shard = np.ascontiguousarray(
            shard.transpose(2, 0, 3, 1)).reshape(HSH, C, WT)  # w-major
        in_maps.append({"xs": shard.astype(BF), **consts})

    global _last_in_maps
    _last_in_maps = in_maps
    res = run_bass_kernel_spmd(nc, in_maps, list(range(NCORES)))

    out = np.empty((B, C, T, H, W), np.float32)
    for core in range(NCORES):
        b, hg = core // 4, core % 4
        o = res.results[core]["out"].reshape(HSH, C, W, T)
        out[b, :, :, hg * HSH:(hg + 1) * HSH, :] = o.transpose(1, 3, 0, 2)
    return out


# revision 16
# speedup vs baseline: 2.1667x; 1.1437x over previous
"""CausalTemporalAttnBlock Trainium2 kernel.

Problem: out = x + Wp @ attn(norm(x)) + bp, where norm is GroupNorm(1 group)
over (c,t,h,w) per batch, attention is causal over t, independent per (b,h,w).
Shapes: x (2, 512, 64, 32, 32) fp32; four (512,512) weights + biases.

Strategy (8 NeuronCores, zero communication except a 8-byte AllReduce for
the GroupNorm stats):
  - core i handles batch i//4, h-rows [8*(i%4), 8*(i%4)+8), all w: 256 (h,w)
    locations per core.
  - Whole matmul datapath in bf16 (fp32 PSUM accumulation): full-rate PE
    streaming (fp32 is half rate) and fast weight load. Host quantizes x
    and the folded weights to bf16 (RNE); error stays ~1e-3 vs the 2e-2
    budget.
  - Host folds gamma/beta into the projection weights:
        q = r*(Aq @ x) + (cq - mu*r*uq),  Aq = wq*diag(gamma) (q pre-scaled
    by 1/sqrt(c)), uq = wq@gamma, cq = bq + wq@beta; same for k. The V-path
    affine is folded all the way into the P-projection eviction:
        out = x + r*(Wp @ attn @ (Av x)^T) + (Wp @ dv + bp), dv = cv - mu*r*uv
    (softmax rows sum to 1, so the V bias passes through attention as a
    per-channel constant). mu, r=rstd computed on device (AllReduce of
    per-batch sum/sumsq across the 4 cores of each batch).
  - Host re-lays the shard w-major: [8 h-rows][512 c][32 w * 64 t], so one
    attention group (8 w-locations) is a contiguous 512-column slice.
  - Locations are processed in PAIRS sharing the 128-wide stationary
    operand: VT for 2 locations in one matmul (x-pair stationary, Wv
    moving), scores S^T = K^T Q per pair as one [128,128] matmul (cross
    terms masked off with the causal mask), and AV as a full-K [128,128]
    matmul per pair (masked zeros in attn kill the cross contributions).
  - No max-subtraction in softmax (scores are O(1)); causal+pair mask is a
    0/1 multiply after exp; normalization by 1/rowsum via ones-matmul
    reductions/broadcasts on the PE.
"""

import numpy as np
import ml_dtypes

import concourse.bass as bass
import concourse.tile as tile
from concourse import bacc, mybir
from concourse.bass_utils import run_bass_kernel_spmd

P = 128
B, C, T, H, W = 2, 512, 64, 32, 32
NCORES = 8
HSH = H // 4          # 8 h-rows per core
CCH = C // P          # 4 c chunks
GRP = 8               # locations per attention group
NGRP = W // GRP       # 4 groups per h-row block
NPR = GRP // 2        # 4 location-pairs per group
WT = W * T            # 2048 free columns per (h-row, c) plane
EPS = 1e-6

f32 = mybir.dt.float32
bf16 = mybir.dt.bfloat16
AX = mybir.AxisListType.X
ALU = mybir.AluOpType
AF = mybir.ActivationFunctionType
BF = ml_dtypes.bfloat16


def build_nc(num_cores=NCORES, nblk=HSH, norm_n=None, replica_groups=None,
             use_collective=True):
    if norm_n is None:
        norm_n = C * T * H * W
    if replica_groups is None:
        replica_groups = [[0, 1, 2, 3], [4, 5, 6, 7]]
    nc = bacc.Bacc("TRN2", target_bir_lowering=False, debug=False,
                   num_devices=num_cores)

    xs = nc.declare_dram_parameter("xs", [nblk, C, WT], bf16, isOutput=False)
    wts = {}
    for nm in ("q", "k", "v", "p"):
        wts[nm] = nc.declare_dram_parameter(f"w{nm}t", [C, C], bf16,
                                            isOutput=False)
    ucol = nc.declare_dram_parameter("ucol", [P, 3 * CCH], f32, isOutput=False)
    ccol = nc.declare_dram_parameter("ccol", [P, 3 * CCH], f32, isOutput=False)
    bpcol = nc.declare_dram_parameter("bpcol", [P, CCH], f32, isOutput=False)
    maskp = nc.declare_dram_parameter("maskt", [P, NPR * P], bf16,
                                      isOutput=False)
    ones_row_f = nc.declare_dram_parameter("ones_row_f", [1, P], f32,
                                           isOutput=False)
    ones_col_b = nc.declare_dram_parameter("ones_col_b", [P, 1], bf16,
                                           isOutput=False)
    ones_mat_b = nc.declare_dram_parameter("ones_mat_b", [P, P], bf16,
                                           isOutput=False)
    outp = nc.declare_dram_parameter("out", [nblk, C, WT], f32, isOutput=True)
    cc_in = nc.dram_tensor("cc_in", [1, 2], f32)
    cc_out = nc.dram_tensor("cc_out", [1, 2], f32)

    with tile.TileContext(nc) as tc:
        with (
            tc.tile_pool(name="const", bufs=1) as const,
            tc.tile_pool(name="scal", bufs=1) as sc,
            tc.tile_pool(name="statp", bufs=4) as statp,
            tc.tile_pool(name="sqp", bufs=2) as sqp,
            tc.tile_pool(name="xpool", bufs=2) as xpool,
            tc.tile_pool(name="gpool", bufs=8) as gpool,
            tc.tile_pool(name="spool", bufs=2) as spool,
            tc.tile_pool(name="opool", bufs=4) as opool,
            tc.tile_pool(name="pp", bufs=3, space="PSUM") as pp,
            tc.tile_pool(name="pss", bufs=2, space="PSUM") as pss,
            tc.tile_pool(name="scp", bufs=2, space="PSUM") as scp,
            tc.tile_pool(name="psm", bufs=1, space="PSUM") as psm,
        ):
            # ---------- constants ----------
            w_sb = {}
            for nm in ("q", "k", "v", "p"):
                for ci in range(CCH):
                    t = const.tile([P, C], bf16, tag=f"w{nm}{ci}")
                    nc.sync.dma_start(t[:], wts[nm][ci * P:(ci + 1) * P, :])
                    w_sb[nm, ci] = t
            ucol_sb = const.tile([P, 3 * CCH], f32, tag="ucol")
            nc.sync.dma_start(ucol_sb[:], ucol[:])
            ccol_sb = const.tile([P, 3 * CCH], f32, tag="ccol")
            nc.sync.dma_start(ccol_sb[:], ccol[:])
            bpcol_sb = const.tile([P, CCH], f32, tag="bpcol")
            nc.sync.dma_start(bpcol_sb[:], bpcol[:])
            mask_sb = const.tile([P, NPR * P], bf16, tag="maskt")
            nc.sync.dma_start(mask_sb[:], maskp[:])
            ocb_sb = const.tile([P, 1], bf16, tag="ocb")
            nc.sync.dma_start(ocb_sb[:], ones_col_b[:])
            orf_sb = const.tile([1, P], f32, tag="orf")
            nc.sync.dma_start(orf_sb[:], ones_row_f[:])
            omb_sb = const.tile([P, P], bf16, tag="omb")
            nc.sync.dma_start(omb_sb[:], ones_mat_b[:])

            # ---------- stats (sum / sumsq over the whole shard) ----------
            # x-sum via ones-matmuls accumulating in one PSUM bank (PE is
            # idle during the stats phase); sumsq fused into the Square
            # activation's accum_out — DVE does almost no stats work
            ssq = sc.tile([P, nblk * CCH], f32, tag="ssq")
            ps_sum1 = psm.tile([1, 512], f32, tag="psm")
            nt = nblk * CCH
            for blk in range(nblk):
                for ci in range(CCH):
                    xt = statp.tile([P, WT], bf16, tag="xstat")
                    nc.sync.dma_start(xt[:], xs[blk, ci * P:(ci + 1) * P, :])
                    i = blk * CCH + ci
                    for j in range(WT // 512):
                        nc.tensor.matmul(
                            ps_sum1[:], ocb_sb[:],
                            xt[:, j * 512:(j + 1) * 512],
                            start=(i == 0 and j == 0),
                            stop=(i == nt - 1 and j == WT // 512 - 1),
                            skip_group_check=True)
                    sq = sqp.tile([P, WT], bf16, tag="sq")
                    nc.scalar.activation(sq[:], xt[:], AF.Square,
                                         accum_out=ssq[:, i:i + 1])
            st_sb = sc.tile([1, 2], f32, tag="st_sb")
            nc.vector.reduce_sum(out=st_sb[0:1, 0:1], in_=ps_sum1[:], axis=AX)
            nc.gpsimd.reduce_sum(out=st_sb[0:1, 1:2], in_=ssq[:],
                                 axis=mybir.AxisListType.XYZWC)
            nc.gpsimd.dma_start(cc_in[:], st_sb[:])
            if use_collective:
                nc.gpsimd.collective_compute(
                    "AllReduce", ALU.add, replica_groups=replica_groups,
                    ins=[cc_in[:]], outs=[cc_out[:]])
            else:
                nc.gpsimd.dma_start(cc_out[:], cc_in[:])
            stg = sc.tile([1, 2], f32, tag="stg")
            nc.gpsimd.dma_start(stg[:], cc_out[:])

            mean = sc.tile([1, 1], f32, tag="mean")
            nc.scalar.activation(mean[:], stg[:, 0:1], AF.Copy,
                                 bias=0.0, scale=1.0 / norm_n)
            ex2 = sc.tile([1, 1], f32, tag="ex2")
            nc.scalar.activation(ex2[:], stg[:, 1:2], AF.Copy,
                                 bias=0.0, scale=1.0 / norm_n)
            msq = sc.tile([1, 1], f32, tag="msq")
            nc.scalar.activation(msq[:], mean[:], AF.Square)
            varp = sc.tile([1, 1], f32, tag="varp")
            nc.vector.tensor_scalar(varp[:], ex2[:], msq[:], EPS,
                                    ALU.subtract, ALU.add)
            sqv = sc.tile([1, 1], f32, tag="sqv")      # = 1/rstd
            nc.scalar.activation(sqv[:], varp[:], AF.Sqrt)
            rst = sc.tile([1, 1], f32, tag="rst")      # = rstd
            nc.vector.reciprocal(rst[:], sqv[:])
            rmu = sc.tile([1, 1], f32, tag="rmu")      # = rstd*mean
            nc.vector.tensor_scalar(rmu[:], mean[:], rst[:], None, ALU.mult)
            vals = sc.tile([1, 2], f32, tag="vals")
            nc.vector.tensor_copy(vals[:, 0:1], rst[:])
            nc.vector.tensor_copy(vals[:, 1:2], rmu[:])
            # broadcast (rstd, rstd*mean) across 128 partitions via K=1 matmul
            ps_b = psm.tile([P, 512], f32, tag="psm")
            nc.tensor.matmul(ps_b[:, 0:2], orf_sb[:], vals[:],
                             start=True, stop=True)
            rb = sc.tile([P, 2], f32, tag="rb")
            nc.vector.tensor_copy(rb[:], ps_b[:, 0:2])
            # per-(proj,chunk) eviction biases for q,k,v: D = ccol - rmu*ucol
            dcol = sc.tile([P, 3 * CCH], f32, tag="dcol")
            nc.vector.tensor_scalar(dcol[:], ucol_sb[:], rb[:, 1:2], None,
                                    ALU.mult)
            nc.vector.tensor_sub(dcol[:], ccol_sb[:], dcol[:])
            # dp = Wp @ dv + bp  (per-channel constant added at P-eviction)
            dvb = sc.tile([P, CCH], bf16, tag="dvb")
            nc.vector.tensor_copy(dvb[:], dcol[:, 2 * CCH:3 * CCH])
            ps_dp = psm.tile([P, 512], f32, tag="psm")
            for co in range(CCH):
                for ci in range(CCH):
                    nc.tensor.matmul(
                        ps_dp[:, co:co + 1],
                        w_sb["p", ci][:, co * P:(co + 1) * P],
                        dvb[:, ci:ci + 1], start=(co == 0 and ci == 0),
                        stop=(ci == CCH - 1), skip_group_check=True)
            dp = sc.tile([P, CCH], f32, tag="dp")
            nc.vector.tensor_add(dp[:], ps_dp[:, 0:CCH], bpcol_sb[:])

            # ---------- main blocks ----------
            for blk in range(nblk):
                xb = []
                for ci in range(CCH):
                    t = xpool.tile([P, WT], bf16, tag=f"xb{ci}")
                    nc.sync.dma_start(t[:], xs[blk, ci * P:(ci + 1) * P, :])
                    xb.append(t)

                for g in range(NGRP):
                    cs = g * GRP * T          # 512-col slice of this group
                    # ---- Q, K projections: psum[co, (w,t)] over ci ----
                    qk = {}
                    for pi, nm in enumerate(("q", "k")):
                        for co in range(CCH):
                            ps = pp.tile([P, 512], f32, tag="pp")
                            for ci in range(CCH):
                                nc.tensor.matmul(
                                    ps[:], w_sb[nm, ci][:, co * P:(co + 1) * P],
                                    xb[ci][:, cs:cs + 512], start=(ci == 0),
                                    stop=(ci == CCH - 1))
                            t = gpool.tile([P, 512], bf16, tag=f"{nm}g")
                            d = pi * CCH + co
                            # affine eviction on ScalarE (closer to PSUM,
                            # keeps DVE free): t = ps*r + dcol
                            nc.scalar.activation(
                                t[:], ps[:], AF.Identity,
                                bias=dcol[:, d:d + 1], scale=rb[:, 0:1])
                            qk[nm, co] = t

                    # ---- VT (raw): per loc PAIR, [128 (2w,s), 512 co] ----
                    vtp = []
                    for p in range(NPR):
                        ps = pss.tile([P, 512], f32, tag="ppv")
                        for ci in range(CCH):
                            nc.tensor.matmul(
                                ps[:], xb[ci][:, cs + p * P:cs + (p + 1) * P],
                                w_sb["v", ci][:], start=(ci == 0),
                                stop=(ci == CCH - 1))
                        t = gpool.tile([P, 512], bf16, tag="vtg")
                        nc.scalar.copy(t[:], ps[:])
                        vtp.append(t)

                    # ---- scores S^T[(2w,s), (2w,t)] per pair ----
                    # 4 pair-chains share one PSUM bank: the very first matmul
                    # start=True zeroes the bank, later chains' first matmuls
                    # overwrite (has_written cleared) and accumulate over ci.
                    ps_s = scp.tile([P, 512], f32, tag="pss")
                    for p in range(NPR):
                        for ci in range(CCH):
                            nc.tensor.matmul(
                                ps_s[:, p * P:(p + 1) * P],
                                qk["k", ci][:, p * P:(p + 1) * P],
                                qk["q", ci][:, p * P:(p + 1) * P],
                                start=(p == 0 and ci == 0),
                                stop=(ci == CCH - 1), skip_group_check=True)
                    # ---- softmax (no max-subtraction) ----
                    pexp = spool.tile([P, 512], bf16, tag="pexp")
                    nc.scalar.activation(pexp[:], ps_s[:], AF.Exp)
                    pm = spool.tile([P, 512], bf16, tag="pmask")
                    nc.vector.tensor_mul(pm[:], pexp[:], mask_sb[:])
                    # rowsums broadcast to all partitions in one matmul
                    # (all-ones stationary), so the reciprocal runs on 128
                    # lanes instead of one
                    ps_sum = psm.tile([P, 512], f32, tag="psm")
                    nc.tensor.matmul(ps_sum[:], omb_sb[:], pm[:],
                                     start=True, stop=True)
                    rs = spool.tile([P, 512], bf16, tag="rs")
                    with nc.allow_low_precision(
                            reason="bf16 softmax denom fine at 2e-2 target"):
                        nc.vector.reciprocal(rs[:], ps_sum[:])
                    pn = spool.tile([P, 512], bf16, tag="pn")
                    nc.vector.tensor_mul(pn[:], pm[:], rs[:])

                    # ---- AV: O[c,(2w,t)] per pair, full-K (mask zeros kill
                    # the cross-location contributions) ----
                    og = []
                    for ch in range(CCH):
                        ps_o = pp.tile([P, 512], f32, tag="pp")
                        for p in range(NPR):
                            nc.tensor.matmul(
                                ps_o[:, p * P:(p + 1) * P],
                                vtp[p][:, ch * P:(ch + 1) * P],
                                pn[:, p * P:(p + 1) * P],
                                start=(p == 0), stop=True,
                                skip_group_check=True)
                        t = gpool.tile([P, 512], bf16, tag="og")
                        nc.vector.tensor_copy(t[:], ps_o[:])
                        og.append(t)

                    # ---- P-projection + affine + residual ----
                    for co in range(CCH):
                        ps = pp.tile([P, 512], f32, tag="pp")
                        for ci in range(CCH):
                            nc.tensor.matmul(
                                ps[:], w_sb["p", ci][:, co * P:(co + 1) * P],
                                og[ci][:], start=(ci == 0),
                                stop=(ci == CCH - 1))
                        slab = opool.tile([P, 512], f32, tag="oslab")
                        nc.vector.tensor_scalar(
                            slab[:], ps[:], rb[:, 0:1], dp[:, co:co + 1],
                            ALU.mult, ALU.add)
                        nc.vector.tensor_add(slab[:], slab[:],
                                             xb[co][:, cs:cs + 512])
                        nc.sync.dma_start(
                            outp[blk, co * P:(co + 1) * P, cs:cs + 512],
                            slab[:])
    nc.compile()
    return nc


def host_prep(gamma, beta, wq, bq, wk, bk, wv, bv, wp, bp):
    """Fold gamma/beta into weights; build all constant tensors."""
    s = 1.0 / np.sqrt(np.float32(C))
    g = gamma.astype(np.float64)

    def fold(w, bias, scale):
        a = (w.astype(np.float64) * g[None, :]) * scale      # (co, ci)
        u = (w.astype(np.float64) @ g) * scale               # (co,)
        c0 = (bias.astype(np.float64) + w.astype(np.float64) @
              beta.astype(np.float64)) * scale
        return (np.ascontiguousarray(a.T).astype(BF),
                u.astype(np.float32), c0.astype(np.float32))

    aqt, uq, cq = fold(wq, bq, s)
    akt, uk, ck = fold(wk, bk, 1.0)
    avt, uv, cv = fold(wv, bv, 1.0)
    apt = np.ascontiguousarray(wp.T.astype(np.float32)).astype(BF)

    ucol = np.empty((P, 3 * CCH), np.float32)
    ccol = np.empty((P, 3 * CCH), np.float32)
    for pi, (u, c0) in enumerate(((uq, cq), (uk, ck), (uv, cv))):
        for ch in range(CCH):
            ucol[:, pi * CCH + ch] = u[ch * P:(ch + 1) * P]
            ccol[:, pi * CCH + ch] = c0[ch * P:(ch + 1) * P]
    bpcol = np.empty((P, CCH), np.float32)
    for ch in range(CCH):
        bpcol[:, ch] = bp[ch * P:(ch + 1) * P]

    # pair mask [128, 4*128]: diag 64x64 halves get causal triu (s<=t),
    # off-diag (cross-location) halves are zero; identical per pair.
    tri = np.triu(np.ones((T, T), np.float32))
    blkm = np.zeros((P, P), np.float32)
    blkm[0:T, 0:T] = tri
    blkm[T:2 * T, T:2 * T] = tri
    maskt = np.tile(blkm, (1, NPR))

    consts = {
        "wqt": aqt, "wkt": akt, "wvt": avt, "wpt": apt,
        "ucol": ucol, "ccol": ccol, "bpcol": bpcol,
        "maskt": maskt.astype(BF),
        "ones_row_f": np.ones((1, P), np.float32),
        "ones_col_b": np.ones((P, 1), BF),
        "ones_mat_b": np.ones((P, P), BF),
    }
    return consts


_NC_CACHE = {}


def kernel(x, gamma, beta, wq, bq, wk, bk, wv, bv, wp, bp):
    x = np.asarray(x, np.float32)
    args = [np.asarray(a, np.float32) for a in
            (gamma, beta, wq, bq, wk, bk, wv, bv, wp, bp)]
    consts = host_prep(*args)

    if "nc" not in _NC_CACHE:
        _NC_CACHE["nc"] = build_nc()
    nc = _NC_CACHE["nc"]

    in_maps = []
    for core in range(NCORES):
        b, hg = core // 4, core % 4
        shard = x[b, :, :, hg * HSH:(hg + 1) * HSH, :]        # (C,T,HSH,W)
        shard = np.ascontiguousarray(
            shard.transpose(2, 0, 3, 1)).reshape(HSH, C, WT)  # w-major
        in_maps.append({"xs": shard.astype(BF), **consts})

    global _last_in_maps
    _last_in_maps = in_maps
    res = run_bass_kernel_spmd(nc, in_maps, list(range(NCORES)))

    out = np.empty((B, C, T, H, W), np.float32)
    for core in range(NCORES):
        b, hg = core // 4, core % 4
        o = res.results[core]["out"].reshape(HSH, C, W, T)
        out[b, :, :, hg * HSH:(hg + 1) * HSH, :] = o.transpose(1, 3, 0, 2)
    return out


# revision 19
# speedup vs baseline: 2.6689x; 1.2318x over previous
"""CausalTemporalAttnBlock Trainium2 kernel.

Problem: out = x + Wp @ attn(norm(x)) + bp, where norm is GroupNorm(1 group)
over (c,t,h,w) per batch, attention is causal over t, independent per (b,h,w).
Shapes: x (2, 512, 64, 32, 32) fp32; four (512,512) weights + biases.

Strategy (8 NeuronCores, zero communication except a 8-byte AllReduce for
the GroupNorm stats):
  - core i handles batch i//4, h-rows [8*(i%4), 8*(i%4)+8), all w: 256 (h,w)
    locations per core.
  - Whole matmul datapath in bf16 (fp32 PSUM accumulation): full-rate PE
    streaming (fp32 is half rate) and fast weight load. Host quantizes x
    and the folded weights to bf16 (RNE); error stays ~1e-3 vs the 2e-2
    budget.
  - Host folds gamma/beta into the projection weights:
        q = r*(Aq @ x) + (cq - mu*r*uq),  Aq = wq*diag(gamma) (q pre-scaled
    by 1/sqrt(c)), uq = wq@gamma, cq = bq + wq@beta; same for k. The V-path
    affine is folded all the way into the P-projection eviction:
        out = x + r*(Wp @ attn @ (Av x)^T) + (Wp @ dv + bp), dv = cv - mu*r*uv
    (softmax rows sum to 1, so the V bias passes through attention as a
    per-channel constant). mu, r=rstd computed on device (AllReduce of
    per-batch sum/sumsq across the 4 cores of each batch).
  - Host re-lays the shard w-major: [8 h-rows][512 c][32 w * 64 t], so one
    attention group (8 w-locations) is a contiguous 512-column slice.
  - Locations are processed in PAIRS sharing the 128-wide stationary
    operand: VT for 2 locations in one matmul (x-pair stationary, Wv
    moving), scores S^T = K^T Q per pair as one [128,128] matmul (cross
    terms masked off with the causal mask), and AV as a full-K [128,128]
    matmul per pair (masked zeros in attn kill the cross contributions).
  - No max-subtraction in softmax (scores are O(1)); causal+pair mask is a
    0/1 multiply after exp; normalization by 1/rowsum via ones-matmul
    reductions/broadcasts on the PE.
"""

import numpy as np
import ml_dtypes

import concourse.bass as bass
import concourse.tile as tile
from concourse import bacc, mybir
from concourse.bass_utils import run_bass_kernel_spmd

P = 128
B, C, T, H, W = 2, 512, 64, 32, 32
NCORES = 8
HSH = H // 4          # 8 h-rows per core
CCH = C // P          # 4 c chunks
GRP = 8               # locations per attention group
NGRP = W // GRP       # 4 groups per h-row block
NPR = GRP // 2        # 4 location-pairs per group
WT = W * T            # 2048 free columns per (h-row, c) plane
EPS = 1e-6

f32 = mybir.dt.float32
bf16 = mybir.dt.bfloat16
AX = mybir.AxisListType.X
ALU = mybir.AluOpType
AF = mybir.ActivationFunctionType
BF = ml_dtypes.bfloat16


def build_nc(num_cores=NCORES, nblk=HSH, norm_n=None, replica_groups=None,
             use_collective=True):
    if norm_n is None:
        norm_n = C * T * H * W
    if replica_groups is None:
        replica_groups = [[0, 1, 2, 3], [4, 5, 6, 7]]
    nc = bacc.Bacc("TRN2", target_bir_lowering=False, debug=False,
                   num_devices=num_cores)

    xs = nc.declare_dram_parameter("xs", [nblk, C, WT], bf16, isOutput=False)
    wts = {}
    for nm in ("q", "k", "v", "p"):
        wts[nm] = nc.declare_dram_parameter(f"w{nm}t", [C, C], bf16,
                                            isOutput=False)
    ucol = nc.declare_dram_parameter("ucol", [P, 3 * CCH], f32, isOutput=False)
    ccol = nc.declare_dram_parameter("ccol", [P, 3 * CCH], f32, isOutput=False)
    bpcol = nc.declare_dram_parameter("bpcol", [P, CCH], f32, isOutput=False)
    maskp = nc.declare_dram_parameter("maskt", [P, NPR * P], bf16,
                                      isOutput=False)
    ones_row_f = nc.declare_dram_parameter("ones_row_f", [1, P], f32,
                                           isOutput=False)
    ones_col_b = nc.declare_dram_parameter("ones_col_b", [P, 1], bf16,
                                           isOutput=False)
    ones_mat_b = nc.declare_dram_parameter("ones_mat_b", [P, P], bf16,
                                           isOutput=False)
    outp = nc.declare_dram_parameter("out", [nblk, C, WT], f32, isOutput=True)
    cc_in = nc.dram_tensor("cc_in", [1, 2], f32)
    cc_out = nc.dram_tensor("cc_out", [1, 2], f32)

    with tile.TileContext(nc) as tc:
        with (
            tc.tile_pool(name="const", bufs=1) as const,
            tc.tile_pool(name="scal", bufs=1) as sc,
            tc.tile_pool(name="statp", bufs=4) as statp,
            tc.tile_pool(name="sqp", bufs=2) as sqp,
            tc.tile_pool(name="xpool", bufs=2) as xpool,
            tc.tile_pool(name="gpool", bufs=8) as gpool,
            tc.tile_pool(name="spool", bufs=2) as spool,
            tc.tile_pool(name="opool", bufs=4) as opool,
            tc.tile_pool(name="pp", bufs=3, space="PSUM") as pp,
            tc.tile_pool(name="pss", bufs=2, space="PSUM") as pss,
            tc.tile_pool(name="scp", bufs=2, space="PSUM") as scp,
            tc.tile_pool(name="psm", bufs=1, space="PSUM") as psm,
        ):
            # ---------- constants ----------
            w_sb = {}
            for nm in ("q", "k", "v", "p"):
                for ci in range(CCH):
                    t = const.tile([P, C], bf16, tag=f"w{nm}{ci}")
                    nc.sync.dma_start(t[:], wts[nm][ci * P:(ci + 1) * P, :])
                    w_sb[nm, ci] = t
            ucol_sb = const.tile([P, 3 * CCH], f32, tag="ucol")
            nc.sync.dma_start(ucol_sb[:], ucol[:])
            ccol_sb = const.tile([P, 3 * CCH], f32, tag="ccol")
            nc.sync.dma_start(ccol_sb[:], ccol[:])
            bpcol_sb = const.tile([P, CCH], f32, tag="bpcol")
            nc.sync.dma_start(bpcol_sb[:], bpcol[:])
            mask_sb = const.tile([P, NPR * P], bf16, tag="maskt")
            nc.sync.dma_start(mask_sb[:], maskp[:])
            ocb_sb = const.tile([P, 1], bf16, tag="ocb")
            nc.sync.dma_start(ocb_sb[:], ones_col_b[:])
            orf_sb = const.tile([1, P], f32, tag="orf")
            nc.sync.dma_start(orf_sb[:], ones_row_f[:])
            omb_sb = const.tile([P, P], bf16, tag="omb")
            nc.sync.dma_start(omb_sb[:], ones_mat_b[:])

            # ---------- stats (sum / sumsq over the whole shard) ----------
            # x-sum via ones-matmuls accumulating in one PSUM bank (PE is
            # idle during the stats phase); sumsq fused into the Square
            # activation's accum_out — DVE does almost no stats work
            ssq = sc.tile([P, nblk * CCH], f32, tag="ssq")
            ps_sum1 = psm.tile([1, 512], f32, tag="psm")
            nt = nblk * CCH
            for blk in range(nblk):
                for ci in range(CCH):
                    xt = statp.tile([P, WT], bf16, tag="xstat")
                    nc.sync.dma_start(xt[:], xs[blk, ci * P:(ci + 1) * P, :])
                    i = blk * CCH + ci
                    for j in range(WT // 512):
                        nc.tensor.matmul(
                            ps_sum1[:], ocb_sb[:],
                            xt[:, j * 512:(j + 1) * 512],
                            start=(i == 0 and j == 0),
                            stop=(i == nt - 1 and j == WT // 512 - 1),
                            skip_group_check=True)
                    sq = sqp.tile([P, WT], bf16, tag="sq")
                    if i % 3 == 2:
                        # spread the square+reduce work across DVE too so the
                        # stats phase isn't paced by ScalarE alone
                        nc.vector.tensor_mul(sq[:], xt[:], xt[:])
                        nc.vector.reduce_sum(out=ssq[:, i:i + 1], in_=sq[:],
                                             axis=AX)
                    else:
                        nc.scalar.activation(sq[:], xt[:], AF.Square,
                                             accum_out=ssq[:, i:i + 1])
            st_sb = sc.tile([1, 2], f32, tag="st_sb")
            nc.vector.reduce_sum(out=st_sb[0:1, 0:1], in_=ps_sum1[:], axis=AX)
            nc.gpsimd.reduce_sum(out=st_sb[0:1, 1:2], in_=ssq[:],
                                 axis=mybir.AxisListType.XYZWC)
            nc.gpsimd.dma_start(cc_in[:], st_sb[:])
            if use_collective:
                nc.gpsimd.collective_compute(
                    "AllReduce", ALU.add, replica_groups=replica_groups,
                    ins=[cc_in[:]], outs=[cc_out[:]])
            else:
                nc.gpsimd.dma_start(cc_out[:], cc_in[:])
            stg = sc.tile([1, 2], f32, tag="stg")
            nc.gpsimd.dma_start(stg[:], cc_out[:])

            mean = sc.tile([1, 1], f32, tag="mean")
            nc.scalar.activation(mean[:], stg[:, 0:1], AF.Copy,
                                 bias=0.0, scale=1.0 / norm_n)
            ex2 = sc.tile([1, 1], f32, tag="ex2")
            nc.scalar.activation(ex2[:], stg[:, 1:2], AF.Copy,
                                 bias=0.0, scale=1.0 / norm_n)
            msq = sc.tile([1, 1], f32, tag="msq")
            nc.scalar.activation(msq[:], mean[:], AF.Square)
            varp = sc.tile([1, 1], f32, tag="varp")
            nc.vector.tensor_scalar(varp[:], ex2[:], msq[:], EPS,
                                    ALU.subtract, ALU.add)
            sqv = sc.tile([1, 1], f32, tag="sqv")      # = 1/rstd
            nc.scalar.activation(sqv[:], varp[:], AF.Sqrt)
            rst = sc.tile([1, 1], f32, tag="rst")      # = rstd
            nc.vector.reciprocal(rst[:], sqv[:])
            rmu = sc.tile([1, 1], f32, tag="rmu")      # = rstd*mean
            nc.vector.tensor_scalar(rmu[:], mean[:], rst[:], None, ALU.mult)
            vals = sc.tile([1, 3], f32, tag="vals")
            nc.vector.tensor_copy(vals[:, 0:1], rst[:])
            nc.vector.tensor_copy(vals[:, 1:2], rmu[:])
            nc.vector.tensor_copy(vals[:, 2:3], sqv[:])
            # broadcast (rstd, rstd*mean, 1/rstd) across partitions (K=1 mm)
            ps_b = psm.tile([P, 512], f32, tag="psm")
            nc.tensor.matmul(ps_b[:, 0:3], orf_sb[:], vals[:],
                             start=True, stop=True)
            rb = sc.tile([P, 3], f32, tag="rb")
            nc.vector.tensor_copy(rb[:], ps_b[:, 0:3])
            # all-(1/r) stationary for the softmax denominator matmul: the
            # rowsum matmul then directly yields Z/r, whose reciprocal is the
            # r/Z factor applied at the AV eviction
            oiv = sc.tile([P, P], bf16, tag="oiv")
            nc.vector.tensor_scalar(oiv[:], omb_sb[:], rb[:, 2:3], None,
                                    ALU.mult)
            # per-(proj,chunk) eviction biases for q,k,v: D = ccol - rmu*ucol
            dcol = sc.tile([P, 3 * CCH], f32, tag="dcol")
            nc.vector.tensor_scalar(dcol[:], ucol_sb[:], rb[:, 1:2], None,
                                    ALU.mult)
            nc.vector.tensor_sub(dcol[:], ccol_sb[:], dcol[:])
            # dp = Wp @ dv + bp  (per-channel constant added at P-eviction)
            dvb = sc.tile([P, CCH], bf16, tag="dvb")
            nc.vector.tensor_copy(dvb[:], dcol[:, 2 * CCH:3 * CCH])
            ps_dp = psm.tile([P, 512], f32, tag="psm")
            for co in range(CCH):
                for ci in range(CCH):
                    nc.tensor.matmul(
                        ps_dp[:, co:co + 1],
                        w_sb["p", ci][:, co * P:(co + 1) * P],
                        dvb[:, ci:ci + 1], start=(co == 0 and ci == 0),
                        stop=(ci == CCH - 1), skip_group_check=True)
            dp = sc.tile([P, CCH], f32, tag="dp")
            nc.vector.tensor_add(dp[:], ps_dp[:, 0:CCH], bpcol_sb[:])

            # ---------- main blocks ----------
            # One-group software pipeline: stage 1 of group g (projections,
            # scores, exp+mask) is emitted before stage 2 of group g-1
            # (rowsum, AV, P, out), so the softmax latency of g hides under
            # the projection matmuls of g and AV/P of g-1 — the in-order PE
            # queue never waits on ScalarE/DVE.

            def stage1(xb, cs):
                # Q, K projections: psum[co, (w,t)] over ci
                qk = {}
                for pi, nm in enumerate(("q", "k")):
                    for co in range(CCH):
                        ps = pp.tile([P, 512], f32, tag="pp")
                        for ci in range(CCH):
                            nc.tensor.matmul(
                                ps[:], w_sb[nm, ci][:, co * P:(co + 1) * P],
                                xb[ci][:, cs:cs + 512], start=(ci == 0),
                                stop=(ci == CCH - 1))
                        t = gpool.tile([P, 512], bf16, tag=f"{nm}g")
                        d = pi * CCH + co
                        # affine eviction on ScalarE (closer to PSUM, keeps
                        # DVE free): t = ps*r + dcol
                        nc.scalar.activation(
                            t[:], ps[:], AF.Identity,
                            bias=dcol[:, d:d + 1], scale=rb[:, 0:1])
                        qk[nm, co] = t

                # VT (raw): per loc PAIR, [128 (2w,s), 512 co]
                vtp = []
                for p in range(NPR):
                    ps = pss.tile([P, 512], f32, tag="ppv")
                    for ci in range(CCH):
                        nc.tensor.matmul(
                            ps[:], xb[ci][:, cs + p * P:cs + (p + 1) * P],
                            w_sb["v", ci][:], start=(ci == 0),
                            stop=(ci == CCH - 1))
                    t = gpool.tile([P, 512], bf16, tag="vtg")
                    nc.scalar.copy(t[:], ps[:])
                    vtp.append(t)

                # scores S^T[(2w,s), (2w,t)] per pair; 4 pair-chains share
                # one PSUM bank: the very first matmul start=True zeroes the
                # bank, later chains' first matmuls overwrite (has_written
                # cleared) and accumulate over ci.
                ps_s = scp.tile([P, 512], f32, tag="pss")
                for p in range(NPR):
                    for ci in range(CCH):
                        nc.tensor.matmul(
                            ps_s[:, p * P:(p + 1) * P],
                            qk["k", ci][:, p * P:(p + 1) * P],
                            qk["q", ci][:, p * P:(p + 1) * P],
                            start=(p == 0 and ci == 0),
                            stop=(ci == CCH - 1), skip_group_check=True)
                # unnormalized masked softmax numerator (normalization is
                # folded into the AV eviction as r/Z)
                pexp = spool.tile([P, 512], bf16, tag="pexp")
                nc.scalar.activation(pexp[:], ps_s[:], AF.Exp)
                pm = spool.tile([P, 512], bf16, tag="pmask")
                nc.vector.tensor_mul(pm[:], pexp[:], mask_sb[:])
                return vtp, pm

            def stage2(st):
                xb, cs, blk, vtp, pm = st
                # rowsum matmul with all-(1/r) stationary => Z/r, broadcast
                # across partitions; fast-approx reciprocal gives r/Z with
                # ~18 good bits, plenty for the bf16 og tiles
                ps_sum = psm.tile([P, 512], f32, tag="psm")
                nc.tensor.matmul(ps_sum[:], oiv[:], pm[:],
                                 start=True, stop=True)
                rz = spool.tile([P, 512], f32, tag="rz")
                nc.vector.reciprocal_approx_fast(out=rz[:], in_=ps_sum[:])

                # AV: O[c,(2w,t)] per pair, full-K (mask zeros kill the
                # cross-location contributions); eviction applies r/Z
                og = []
                for ch in range(CCH):
                    ps_o = pp.tile([P, 512], f32, tag="pp")
                    for p in range(NPR):
                        nc.tensor.matmul(
                            ps_o[:, p * P:(p + 1) * P],
                            vtp[p][:, ch * P:(ch + 1) * P],
                            pm[:, p * P:(p + 1) * P],
                            start=(p == 0), stop=True,
                            skip_group_check=True)
                    t = gpool.tile([P, 512], bf16, tag="og")
                    nc.vector.tensor_mul(t[:], ps_o[:], rz[:])
                    og.append(t)

                # P-projection + bias + residual
                for co in range(CCH):
                    ps = pp.tile([P, 512], f32, tag="pp")
                    for ci in range(CCH):
                        nc.tensor.matmul(
                            ps[:], w_sb["p", ci][:, co * P:(co + 1) * P],
                            og[ci][:], start=(ci == 0),
                            stop=(ci == CCH - 1))
                    slab = opool.tile([P, 512], f32, tag="oslab")
                    nc.vector.tensor_scalar(
                        slab[:], ps[:], dp[:, co:co + 1], None, ALU.add)
                    nc.vector.tensor_add(slab[:], slab[:],
                                         xb[co][:, cs:cs + 512])
                    nc.sync.dma_start(
                        outp[blk, co * P:(co + 1) * P, cs:cs + 512],
                        slab[:])

            pending = None
            for blk in range(nblk):
                xb = []
                for ci in range(CCH):
                    t = xpool.tile([P, WT], bf16, tag=f"xb{ci}")
                    nc.sync.dma_start(t[:], xs[blk, ci * P:(ci + 1) * P, :])
                    xb.append(t)
                for g in range(NGRP):
                    cs = g * GRP * T          # 512-col slice of this group
                    vtp, pm = stage1(xb, cs)
                    if pending is not None:
                        stage2(pending)
                    pending = (xb, cs, blk, vtp, pm)
            stage2(pending)
    nc.compile()
    return nc


def host_prep(gamma, beta, wq, bq, wk, bk, wv, bv, wp, bp):
    """Fold gamma/beta into weights; build all constant tensors."""
    s = 1.0 / np.sqrt(np.float32(C))
    g = gamma.astype(np.float64)

    def fold(w, bias, scale):
        a = (w.astype(np.float64) * g[None, :]) * scale      # (co, ci)
        u = (w.astype(np.float64) @ g) * scale               # (co,)
        c0 = (bias.astype(np.float64) + w.astype(np.float64) @
              beta.astype(np.float64)) * scale
        return (np.ascontiguousarray(a.T).astype(BF),
                u.astype(np.float32), c0.astype(np.float32))

    aqt, uq, cq = fold(wq, bq, s)
    akt, uk, ck = fold(wk, bk, 1.0)
    avt, uv, cv = fold(wv, bv, 1.0)
    apt = np.ascontiguousarray(wp.T.astype(np.float32)).astype(BF)

    ucol = np.empty((P, 3 * CCH), np.float32)
    ccol = np.empty((P, 3 * CCH), np.float32)
    for pi, (u, c0) in enumerate(((uq, cq), (uk, ck), (uv, cv))):
        for ch in range(CCH):
            ucol[:, pi * CCH + ch] = u[ch * P:(ch + 1) * P]
            ccol[:, pi * CCH + ch] = c0[ch * P:(ch + 1) * P]
    bpcol = np.empty((P, CCH), np.float32)
    for ch in range(CCH):
        bpcol[:, ch] = bp[ch * P:(ch + 1) * P]

    # pair mask [128, 4*128]: diag 64x64 halves get causal triu (s<=t),
    # off-diag (cross-location) halves are zero; identical per pair.
    tri = np.triu(np.ones((T, T), np.float32))
    blkm = np.zeros((P, P), np.float32)
    blkm[0:T, 0:T] = tri
    blkm[T:2 * T, T:2 * T] = tri
    maskt = np.tile(blkm, (1, NPR))

    consts = {
        "wqt": aqt, "wkt": akt, "wvt": avt, "wpt": apt,
        "ucol": ucol, "ccol": ccol, "bpcol": bpcol,
        "maskt": maskt.astype(BF),
        "ones_row_f": np.ones((1, P), np.float32),
        "ones_col_b": np.ones((P, 1), BF),
        "ones_mat_b": np.ones((P, P), BF),
    }
    return consts


_NC_CACHE = {}


def kernel(x, gamma, beta, wq, bq, wk, bk, wv, bv, wp, bp):
    x = np.asarray(x, np.float32)
    args = [np.asarray(a, np.float32) for a in
            (gamma, beta, wq, bq, wk, bk, wv, bv, wp, bp)]
    consts = host_prep(*args)

    if "nc" not in _NC_CACHE:
        _NC_CACHE["nc"] = build_nc()
    nc = _NC_CACHE["nc"]

    in_maps = []
    for core in range(NCORES):
        b, hg = core // 4, core % 4
        shard = x[b, :, :, hg * HSH:(hg + 1) * HSH, :]        # (C,T,HSH,W)
        shard = np.ascontiguousarray(
            shard.transpose(2, 0, 3, 1)).reshape(HSH, C, WT)  # w-major
        in_maps.append({"xs": shard.astype(BF), **consts})

    global _last_in_maps
    _last_in_maps = in_maps
    res = run_bass_kernel_spmd(nc, in_maps, list(range(NCORES)))

    out = np.empty((B, C, T, H, W), np.float32)
    for core in range(NCORES):
        b, hg = core // 4, core % 4
        o = res.results[core]["out"].reshape(HSH, C, W, T)
        out[b, :, :, hg * HSH:(hg + 1) * HSH, :] = o.transpose(1, 3, 0, 2)
    return out
